# revision 27
# baseline (speedup 1.0000x reference)
"""Multi-head attention (B=2, S=2048, D=1024, H=16) on 8 Trainium2 cores.

Sharding: data-parallel over batch (2) x tensor-parallel over heads (16 -> 4
per core). Core c handles batch c//4, heads 4*(c%4) .. 4*(c%4)+3. Each core
computes its heads' Q/K/V projections (column-sliced weights), flash-style
attention with transposed-score layout, and a partial output projection
(row-sliced Wo). Host sums the 4 partials per batch and adds bv@Wo + bo.

v5 design:
  - x and the Q/K/V weights are cast to bf16 on the host. x^T lands in SBUF
    via 32 DMA-XBAR transposes (16-bit only; ~14ns per 16x128 tile on the
    DMA engines) — no PE transposes, no PSUM->SBUF xT copies at all.
  - Q/K projections write bf16 qT/kT (pair-packed [2 heads x 64, S]);
    V-projection writes bf16 va augmented with a ones column whose AV row
    accumulates the softmax denominator.
  - Attention unit u = (i-block, head-pair): j-loop over 16 key tiles;
    scores (row-packed K=64 pair) -> exp on ACT (the true roofline:
    16.8M exps @ 1 elem/lane/cycle @ 1.2GHz ~= 110us) -> AV accumulation.
    Scores for the next unit are emitted inside the current unit's j-loop
    so ACT never stalls at unit boundaries.
  - Unit 0's first 10 j-steps are interleaved into phase 1 (lagging the
    projection stream by 6 tiles), so ACT starts ~40us earlier.
  - Finish work (1/sum broadcast + output projection row-tiles) is spread
    uniformly across subsequent units' j-loops (bc at j==5, one fin at
    j==8 and j==12) to keep the PE dense enough that HAM stays at K=8/8.
  - One PSUM pool, three tags shared by both phases (no pool-close drain):
      short [128,1024] x2: q|k projection pairs, scores
      small [128, 512] x2: v-projection, bc broadcast, out-proj halves
      long  [128,1024] x1: AV accumulators (po0|po1)
  - All SBUF tile free-sizes are 64B multiples: an odd-sized tile mid-pool
    misaligns every later tile's base and costs ~20% on every ACT/DVE op
    and matmul operand read.
"""

import numpy as np

B, S, D, H, DK = 2, 2048, 1024, 16, 64
HPC = 4          # heads per core
HD = HPC * DK    # 256 projected dims per core
P = 128
NB = 512
NCORES = 8

_CACHE = {}


def _install_tile_drain_fix():
    """TileContext._drain_and_barrier piles every outstanding sem wait onto
    one Drain instruction; this walrus build rejects >1 sync wait per
    instruction. Split the extra waits across single-wait NOPs."""
    import concourse.tile as tile
    from concourse.vector_clock import ScopedClock

    if getattr(tile.TileContext, "_ant_drain_fix", False):
        return

    def _drain_and_barrier_split(self, tick_clock, wait_clock):
        drain_inst = self.nc.sync.drain()
        wait_clock.add_sem_waits(
            drain_inst.ins, ScopedClock({None: tick_clock.global_clock})
        )
        waits = list(drain_inst.ins.sync_info.on_wait or [])
        if len(waits) > 1:
            drain_inst.ins.sync_info.on_wait = waits[:1]
            for w in waits[1:]:
                n = self.nc.sync.nop(nofuse=True)
                si = n.ins.sync_info
                if si is None:
                    import bass_rust

                    n.ins.sync_info = bass_rust.SyncInfo(on_wait=[w], on_update=[])
                else:
                    si.on_wait = [w]

        self.nc.all_engine_barrier()
        assert self.sems is not None
        popped = self.nc._tile_sem_poison_stack.pop()
        assert popped is self._sem_poison
        self.nc.clear_and_free_semaphores(list(self.sems.allocated().values()))
        self.nc.all_engine_barrier()

    tile.TileContext._drain_and_barrier = _drain_and_barrier_split
    tile.TileContext._ant_drain_fix = True


def _split_excess_waits(nc):
    """walrus's per-struct sync-wait capacity is small (observed: 1 for the
    self-loading-weight Matmult S3_LW struct, 2 for TPB_CTRL/Drain). Tile's
    wait assignment can leave many waits on one instruction; hoist the excess
    onto NOPs on the same engine immediately before it."""
    import concourse.mybir as mybir

    nid = [0]
    for f in nc.m.functions:
        for bb in f.blocks:
            out = []
            changed = False
            for inst in bb.instructions:
                si = getattr(inst, "sync_info", None)
                waits = list(si.on_wait) if si is not None and si.on_wait else []
                cap = 1
                if len(waits) > cap:
                    extra = waits[cap:]
                    for k in range(0, len(extra), 2):
                        nid[0] += 1
                        out.append(
                            mybir.InstEventSemaphore(
                                name=f"I-waitsplit-{nid[0]}",
                                ins=[],
                                outs=[],
                                sync_info=mybir.SyncInfo(
                                    on_wait=extra[k:k + 2], on_update=[]
                                ),
                                engine=inst.engine,
                            )
                        )
                    si.on_wait = waits[:cap]
                    changed = True
                out.append(inst)
            if changed:
                bb.instructions = out


def _build_program():
    import concourse.bass as bass
    import concourse.mybir as mybir
    from concourse.tile import TileContext

    _install_tile_drain_fix()

    f32 = mybir.dt.float32
    f32r = mybir.dt.float32r
    bf16 = mybir.dt.bfloat16
    Exp = mybir.ActivationFunctionType.Exp

    nc = bass.Bass()

    xbh = nc.dram_tensor("xbh", [S, D], bf16, kind="ExternalInput")
    wq = nc.dram_tensor("wq", [D, HD], bf16, kind="ExternalInput")
    wk = nc.dram_tensor("wk", [D, HD], bf16, kind="ExternalInput")
    wv = nc.dram_tensor("wv", [D, HD], bf16, kind="ExternalInput")
    wo = nc.dram_tensor("wo", [HD, D], f32r, kind="ExternalInput")
    bqt = nc.dram_tensor("bqt", [P, 2], f32, kind="ExternalInput")
    bkt = nc.dram_tensor("bkt", [P, 2], f32, kind="ExternalInput")
    outp = nc.dram_tensor("outp", [S, D], f32, kind="ExternalOutput")

    NDC = D // P      # 8 d-chunks
    NST = S // P      # 16 sequence tiles
    NSB = S // NB     # 4 sequence blocks

    with TileContext(nc) as tc:
        with tc.tile_pool(name="consts", bufs=1) as consts:
            onesg = consts.tile([33, DK], f32r)
            nc.vector.memset(onesg.bitcast(mybir.dt.uint32), 0x3F800000)
            # pre-warm the ACT exp table set (~2.7us) while DMAs run
            wconst = consts.tile([1, 16], f32)
            nc.vector.memset(wconst[:], 1.0)
            warm = consts.tile([1, 16], f32)
            nc.scalar.activation(warm[0:1, 0:1], wconst[0:1, 0:1], Exp)

            # weights ride the scalar-engine HWDGE queue so the x transposes
            # (sync queue) aren't serialized behind them
            wv_sb = consts.tile([P, NDC, HD], bf16)
            nc.scalar.dma_start(wv_sb[:], wv.rearrange("(c p) h -> p c h", p=P))
            wq_sb = consts.tile([P, NDC, HD], bf16)
            nc.scalar.dma_start(wq_sb[:], wq.rearrange("(c p) h -> p c h", p=P))
            wk_sb = consts.tile([P, NDC, HD], bf16)
            nc.scalar.dma_start(wk_sb[:], wk.rearrange("(c p) h -> p c h", p=P))
            bq_sb = consts.tile([P, 16], f32)
            nc.scalar.dma_start(bq_sb[:, 0:2], bqt[:])
            bk_sb = consts.tile([P, 16], f32)
            nc.scalar.dma_start(bk_sb[:, 0:2], bkt[:])
            # wo's DMA is emitted after the x transposes (it's needed ~90us
            # in, and would delay the scalar-queue transposes otherwise)
            wo_sb = consts.tile([P, 2, D], f32r)

            with (
                tc.tile_pool(name="acts", bufs=1) as acts,
                tc.tile_pool(name="ps", bufs=1, space="PSUM") as ps,
            ):
                xT = acts.tile([P, NDC, S], bf16)
                # pair-packed transposed projections: [2 heads x 64, S]
                qT = acts.tile([P, 2, S], bf16)
                kT = acts.tile([P, 2, S], bf16)
                # v augmented with a ones column (row 65 of the AV matmul
                # accumulates the softmax denominator): [s, j-tile, head, 65]
                va = acts.tile([P, NST, HPC, DK + 1], bf16)
                nc.vector.memset(va.bitcast(mybir.dt.uint16), 0x3F80)
                # Wo lhsT: [head-dim pair-chunk, pair, i]
                stack = acts.tile([P, 2, S], f32r)
                # softmax denominators for two heads at partitions 0 and 32;
                # filler rows preset to 1.0 so reciprocal never sees junk
                sums_sb = acts.tile([33, NB], f32)
                nc.vector.memset(sums_sb[:], 1.0)

                # x^T via DMA-XBAR transposes, one per (s-block, d-chunk),
                # split across the two HWDGE queues (sync + scalar) so each
                # s-block's eight chunks land in ~half the serial time
                for sb in range(NSB):
                    for c in range(NDC):
                        eng = nc.sync if c % 2 == 0 else nc.scalar
                        eng.dma_start(
                            xT[:, c, sb * NB:(sb + 1) * NB],
                            xbh[sb * NB:(sb + 1) * NB, c * P:(c + 1) * P],
                            transpose=True,
                        )
                nc.scalar.dma_start(
                    wo_sb[:], wo.rearrange("(c p) d -> p c d", p=P)
                )

                def short(nm):
                    return ps.tile([P, 2 * NB], f32, tag="short", bufs=2, name=nm)

                def small(nm):
                    return ps.tile([P, NB], f32, tag="small", bufs=2, name=nm)

                def long_(nm):
                    return ps.tile([P, 2 * NB], f32, tag="long", bufs=1, name=nm)

                # ---------------- projections -------------------------------
                def emit_vproj(it):
                    vp = small(f"vp{it}")
                    for d in range(NDC):
                        nc.tensor.matmul(
                            vp[:, 0:HD],
                            xT[:, d, it * P:(it + 1) * P],
                            wv_sb[:, d, :],
                            start=(d == 0),
                            stop=(d == NDC - 1),
                        )
                    nc.vector.tensor_copy(
                        out=va[:, it, :, 0:DK],
                        in_=vp[:, 0:HD].rearrange("p (h e) -> p h e", h=HPC),
                    )

                def emit_qk0(sb):
                    # q and k projections for pair 0 of block sb (one 2-bank
                    # tile, q|k halves) — runs in phase 1
                    pq = short(f"pq{sb}")
                    for col, w_sb in ((0, wq_sb), (NB, wk_sb)):
                        for d in range(NDC):
                            nc.tensor.matmul(
                                pq[:, col:col + NB],
                                w_sb[:, d, 0:P],
                                xT[:, d, sb * NB:(sb + 1) * NB],
                                start=(d == 0),
                                stop=(d == NDC - 1),
                            )
                    with nc.allow_low_precision("bf16 q/k feed scores"):
                        nc.vector.tensor_scalar_add(
                            out=qT[:, 0, sb * NB:(sb + 1) * NB],
                            in0=pq[:, 0:NB],
                            scalar1=bq_sb[:, 0:1],
                        )
                        nc.vector.tensor_scalar_add(
                            out=kT[:, 0, sb * NB:(sb + 1) * NB],
                            in0=pq[:, NB:2 * NB],
                            scalar1=bk_sb[:, 0:1],
                        )

                def emit_qk1_half(sb, col):
                    # one of pair 1's q/k projections for block sb on the
                    # 1-bank ring — hosted inside phase-2 j-loops
                    w_sb, b_sb, dT = (
                        (wq_sb, bq_sb, qT) if col == 0 else (wk_sb, bk_sb, kT)
                    )
                    pq = small(f"pq1_{sb}_{col}")
                    for d in range(NDC):
                        nc.tensor.matmul(
                            pq[:],
                            w_sb[:, d, P:2 * P],
                            xT[:, d, sb * NB:(sb + 1) * NB],
                            start=(d == 0),
                            stop=(d == NDC - 1),
                        )
                    with nc.allow_low_precision("bf16 q/k feed scores"):
                        nc.vector.tensor_scalar_add(
                            out=dT[:, 1, sb * NB:(sb + 1) * NB],
                            in0=pq[:],
                            scalar1=b_sb[:, 1:2],
                        )

                # ---------------- attention helpers -------------------------
                # all pair-0 units first: pair-1 q/k projections are hosted
                # inside the pair-0 units' j-loops (phase-2 PE slack)
                units = [(ib, 0) for ib in range(NSB)] + [
                    (ib, 1) for ib in range(NSB)
                ]
                # hosted work: unit u -> (block, q/k half col) at j==1 / j==3
                host_qk = {1: 0, 2: 1, 3: 2, 4: 3}

                def emit_scores(u, j):
                    ib, p = units[u]
                    i0 = ib * NB
                    sc = short(f"sc{u}_{j}")
                    nc.tensor.matmul(
                        sc[:, 0:NB],
                        kT[0:DK, p, j * P:(j + 1) * P],
                        qT[0:DK, p, i0:i0 + NB],
                        tile_position=(0, 0),
                    )
                    nc.tensor.matmul(
                        sc[:, NB:2 * NB],
                        kT[DK:2 * DK, p, j * P:(j + 1) * P],
                        qT[DK:2 * DK, p, i0:i0 + NB],
                        tile_position=(64, 0),
                    )
                    return sc

                def emit_scores_pair(u, j):
                    # boundary variant on the 1-bank ring: the 2-deep score
                    # ring drains to zero depth at unit boundaries (its slot
                    # frees only when exp(u-1, 14) completes), which stalls
                    # the exp stream ~2us; these bypass that ring
                    ib, p = units[u]
                    i0 = ib * NB
                    pair = []
                    for h in range(2):
                        sch = small(f"sc{u}_{j}_{h}")
                        nc.tensor.matmul(
                            sch[:, 0:NB],
                            kT[h * DK:(h + 1) * DK, p, j * P:(j + 1) * P],
                            qT[h * DK:(h + 1) * DK, p, i0:i0 + NB],
                            tile_position=(h * DK, 0),
                        )
                        pair.append(sch)
                    return tuple(pair)

                def emit_bc(u, rcr, po_sbs):
                    # broadcast each head's 1/sumexp across 64 partitions via
                    # rank-1 matmul, then scale the AV numerators into the
                    # Wo lhsT
                    ib, p = units[u]
                    i0 = ib * NB
                    for h in range(2):
                        bct = small(f"bc{u}_{h}")
                        nc.tensor.matmul(
                            bct[0:DK, :],
                            onesg[32 * h:32 * h + 1, :],
                            rcr[32 * h:32 * h + 1, :],
                            tile_position=(32 * h, 0),
                        )
                        nc.vector.tensor_tensor(
                            out=stack[h * DK:(h + 1) * DK, p, i0:i0 + NB],
                            in0=po_sbs[h][:],
                            in1=bct[0:DK, :],
                            op=mybir.AluOpType.mult,
                        )

                def emit_fin(ib, t, tail=False):
                    # output projection for row-tile t of i-block ib, split
                    # into two D-halves on the 1-bank "small" ring; at the
                    # tail the idle score ring is used for every other half
                    # so two halves overlap
                    it = ib * (NB // P) + t
                    for nbi in range(2):
                        fint = (
                            short(f"fin{it}_{nbi}") if (tail and nbi == 1)
                            else small(f"fin{it}_{nbi}")
                        )
                        fin = fint[:, 0:NB]
                        for pch in range(2):
                            nc.tensor.matmul(
                                fin,
                                stack[:, pch, it * P:(it + 1) * P],
                                wo_sb[:, pch, nbi * NB:(nbi + 1) * NB],
                                start=(pch == 0),
                                stop=(pch == 1),
                            )
                        ot = acts.tile(
                            [P, NB], f32, tag="ot", bufs=3, name=f"ot{it}_{nbi}"
                        )
                        nc.vector.tensor_copy(out=ot[:], in_=fin)
                        nc.sync.dma_start(
                            outp[it * P:(it + 1) * P, nbi * NB:(nbi + 1) * NB],
                            ot[:],
                        )

                from collections import deque

                sc_q = deque()
                fin_q = deque()
                state = {"pending": None, "po": None}

                def emit_unit_end(u):
                    # drain accumulators + denominators + reciprocal (DVE)
                    po = state["po"]
                    po_sbs = []
                    for h in range(2):
                        po_sb = acts.tile(
                            [DK, NB], f32, tag="posb", bufs=4, name=f"posb{u}_{h}"
                        )
                        nc.vector.tensor_copy(
                            out=po_sb[:], in_=po[0:DK, h * NB:(h + 1) * NB]
                        )
                        po_sbs.append(po_sb)
                        nc.vector.tensor_copy(
                            out=sums_sb[32 * h:32 * h + 1, :],
                            in_=po[DK:DK + 1, h * NB:(h + 1) * NB],
                        )
                    rcr = acts.tile(
                        [33, NB], f32r, tag="rcr", bufs=2, name=f"rcr{u}"
                    )
                    with nc.allow_low_precision("fp22 recip feeds f32r matmul"):
                        nc.vector.reciprocal(out=rcr[:], in_=sums_sb[:])
                    state["pending"] = (u, rcr, po_sbs)

                def emit_step(u, j):
                    # one attention j-step of unit u
                    ib, p = units[u]
                    if j == 0:
                        state["po"] = long_(f"po{u}")
                    po = state["po"]
                    sc = sc_q.popleft()
                    ex = acts.tile(
                        [P, 2 * NB], bf16, tag="ex", bufs=3, name=f"ex{u}_{j}"
                    )
                    if isinstance(sc, tuple):
                        for h in range(2):
                            nc.scalar.activation(
                                ex[:, h * NB:(h + 1) * NB], sc[h][:, 0:NB],
                                Exp, scale=0.125,
                            )
                    else:
                        nc.scalar.activation(ex[:], sc[:], Exp, scale=0.125)
                    nj = j + 2
                    if nj < NST:
                        sc_q.append(emit_scores(u, nj))
                    elif u + 1 < 8:
                        if nj - NST == 0:
                            sc_q.append(emit_scores_pair(u + 1, 0))
                        else:
                            sc_q.append(emit_scores(u + 1, nj - NST))
                    for h in range(2):
                        nc.tensor.matmul(
                            po[0:DK + 1, h * NB:(h + 1) * NB],
                            va[:, j, 2 * p + h, :],
                            ex[:, h * NB:(h + 1) * NB],
                            start=(j == 0),
                            stop=(j == NST - 1),
                        )
                    # hosted pair-1 q/k projections (phase-2 PE slack)
                    if u in host_qk and j in (1, 3):
                        emit_qk1_half(host_qk[u], 0 if j == 1 else 1)
                    # spread previous-unit finish work across this j-loop
                    if state["pending"] is not None and j == 5:
                        pu, rcr, po_sbs = state["pending"]
                        emit_bc(pu, rcr, po_sbs)
                        if units[pu][1] == 1:
                            for t in range(NB // P):
                                fin_q.append((units[pu][0], t))
                        state["pending"] = None
                    elif j in (7, 9, 11, 13) and fin_q:
                        emit_fin(*fin_q.popleft())
                    if j == NST - 1:
                        emit_unit_end(u)

                # ---------------- fused emission ----------------------------
                # phase 1 with unit 0's j-steps interleaved; qk windows host
                # two steps so the exp stream doesn't starve during them.
                # sc(0, j) may only be emitted after qk0(j//4), hence the
                # lag-4 schedule with double steps at projection tiles.
                steps = {4: [0], 5: [1], 7: [2, 3], 8: [4], 9: [5],
                         11: [6, 7], 12: [8], 13: [9], 15: [10, 11]}
                for it in range(NST):
                    emit_vproj(it)
                    if it % 4 == 3:
                        emit_qk0(it // 4)
                        if it == 3:
                            sc_q.append(emit_scores(0, 0))
                            sc_q.append(emit_scores(0, 1))
                    for j in steps.get(it, []):
                        emit_step(0, j)
                # unit 0 finishes, then units 1..7
                for j in range(12, NST):
                    emit_step(0, j)
                for u in range(1, 8):
                    for j in range(NST):
                        emit_step(u, j)
                # tail: finish of the last unit
                pu, rcr, po_sbs = state["pending"]
                emit_bc(pu, rcr, po_sbs)
                while fin_q:
                    emit_fin(*fin_q.popleft(), tail=True)
                for t in range(NB // P):
                    emit_fin(units[pu][0], t, tail=True)

    _split_excess_waits(nc)
    return nc


def _get_program():
    if "nc" not in _CACHE:
        _CACHE["nc"] = _build_program()
    return _CACHE["nc"]


def kernel(x, Wq, bq, Wk, bk, Wv, bv, Wo, bo, _trace=False):
    import ml_dtypes
    from concourse.bass_utils import run_bass_kernel_spmd

    bft = np.dtype(ml_dtypes.bfloat16)
    x = np.asarray(x, dtype=np.float32)
    Wq = np.asarray(Wq, dtype=np.float32)
    Wk = np.asarray(Wk, dtype=np.float32)
    Wv = np.asarray(Wv, dtype=np.float32)
    Wo = np.asarray(Wo, dtype=np.float32)
    bq = np.asarray(bq, dtype=np.float32)
    bk = np.asarray(bk, dtype=np.float32)
    bv = np.asarray(bv, dtype=np.float32)
    bo = np.asarray(bo, dtype=np.float32)

    in_maps = []
    for c in range(NCORES):
        b = c // 4
        cs = (c % 4) * HD
        in_maps.append({
            "xbh": np.ascontiguousarray(x[b].astype(bft)),
            "wq": np.ascontiguousarray(Wq[:, cs:cs + HD].astype(bft)),
            "wk": np.ascontiguousarray(Wk[:, cs:cs + HD].astype(bft)),
            "wv": np.ascontiguousarray(Wv[:, cs:cs + HD].astype(bft)),
            "wo": np.ascontiguousarray(Wo[cs:cs + HD, :]),
            "bqt": np.ascontiguousarray(bq[cs:cs + HD].reshape(2, P).T),
            "bkt": np.ascontiguousarray(bk[cs:cs + HD].reshape(2, P).T),
        })

    nc = _get_program()
    res = run_bass_kernel_spmd(
        nc, in_maps, core_ids=list(range(NCORES)), trace=_trace
    )

    cvec = (bv @ Wo + bo).astype(np.float32)
    out = np.empty((B, S, D), dtype=np.float32)
    for b in range(B):
        acc = res.results[4 * b]["outp"].astype(np.float64)
        for c in range(4 * b + 1, 4 * b + 4):
            acc = acc + res.results[c]["outp"]
        out[b] = (acc + cvec).astype(np.float32)

    if _trace:
        _CACHE["last_results"] = res
    return out
